# revision 31
# baseline (speedup 1.0000x reference)
"""AdaConv2D Trainium2 Bass kernel.

Problem (per sample): instance-norm(x) -> grouped 3x3 conv (128 groups,
2ch/group, per-sample weights) -> grouped 1x1 conv -> +bias.
B=8, Cin=Cout=256, H=W=128.

Strategy: pure data-parallel, 1 sample per NeuronCore (8 cores).

Per-core design (v3):
  - Host pads x to [256, 130, 130] bf16 with zero borders.  Each input
    chunk DMA lands *directly* in the padded SBUF image xnp (one
    contiguous descriptor per partition) — no on-chip ingest copy.
    Input rides the sync + gpsimd queues (the scalar engine is kept free
    for ACT stats so its DMA issues never stall behind compute).
  - Stats (per half, 8 chunks): per-channel sum via tensor_reduce-XY
    into per-chunk slots (DVE x5, GpSimd tensor_scalar+accum x2, the
    DVE also handles the last chunk); sum-of-squares via ACT
    Square+accum (7 chunks) and DVE TT-square+reduce (last chunk).
    Pad zeros don't disturb the sums.  DVE reductions are 1x by HW
    ucode (PERF_ONE), so the work is spread over three engines.
  - The 1x1 conv is folded into the 3x3 weights; instance norm is folded
    into weights + bias (border cells hold the per-channel mean so
    (border - mean) * scale = 0 matches the reference's zero padding).
  - Weights: 2 PSUM-accumulated permutation matmuls per half move
    group-layout weights to channel-partition layout (parity folded into
    the permutation); one broadcast tensor_tensor per half expands them
    into the dense block-diagonal lhsT (bf16); a tensor_scalar applies
    the norm scale after stats resolve.  h0's finalize+scale chain is
    high-priority and h1's DVE work is dependency-pinned behind it so
    the scheduler cannot interleave h1 into the critical path.
  - Conv: 9 shifted block-diagonal 128x128 bf16 matmuls per 4-row PSUM
    tile, channels on partitions (2 halves).  A dummy-matmul burst gated
    on the last h0 chunk warms the PE clock (HAM) so the conv streams at
    2.4 GHz.
  - Output staged bf16 (host converts back to f32); out-DMAs round-robin
    across sync/scalar/gpsimd queues; the last superblock drains in
    4-row stages on the two HWDGE queues to shorten the tail.
"""

import os
import sys

sys.path.insert(0, "/opt/trn_rl_repo")

from contextlib import ExitStack

# bisection flags (default 0 = full-speed path)
_F_ALL_ACT_SQ = os.environ.get("ADK_ALL_ACT_SQ", "0") == "1"
_F_OUT_GPSIMD = os.environ.get("ADK_OUT_GPSIMD", "0") == "1"
_F_NO_STRIDE0 = os.environ.get("ADK_NO_STRIDE0", "0") == "1"
_F_NO_GPS_SUM = os.environ.get("ADK_NO_GPS_SUM", "0") == "1"

import numpy as np
import ml_dtypes

from concourse import bacc, bass, mybir, tile
from concourse.bass_utils import run_bass_kernel_spmd

F32 = mybir.dt.float32
BF16 = mybir.dt.bfloat16
AX = mybir.AxisListType
OP = mybir.AluOpType
ACTF = mybir.ActivationFunctionType

C = 256          # channels (per sample)
H = W = 128      # spatial
P = 128          # partitions
HP = H + 2       # padded rows/cols (130)
NHF = 2          # channel halves
NPIX = H * W
EPS = 1e-7

# padded-row chunk splits; small leading chunks start the stats
# pipeline as early as possible
CK = [(0, 8), (8, 16), (16, 32), (32, 48), (48, 64), (64, 80),
      (80, 96), (96, 112), (112, 130)]
NCHUNK = len(CK)

ROWS_PER_MM = 4                   # output rows per psum tile (4*128=512)
SB_TILES = 4                      # psum tiles per superblock
SB_ROWS = ROWS_PER_MM * SB_TILES  # 16 rows per superblock
NSB = H // SB_ROWS                # 8 superblocks per half

_CACHED = {}


def build_nc():
    nc = bacc.Bacc(trn_type="TRN2")

    x_ext = nc.declare_dram_parameter("x", [C, HP, HP], BF16, isOutput=False)
    dw_ext = nc.declare_dram_parameter("dw_kernels", [C, 2, 3, 3], F32, isOutput=False)
    pw_ext = nc.declare_dram_parameter("pw_kernels", [C, 2, 1, 1], F32, isOutput=False)
    b_ext = nc.declare_dram_parameter("biases", [C], F32, isOutput=False)
    out_ext = nc.declare_dram_parameter("out", [C, H, W], BF16, isOutput=True)

    with tile.TileContext(nc) as tc, ExitStack() as ctx:
        const_pool = ctx.enter_context(tc.tile_pool(name="const", bufs=1))
        sq_pool = ctx.enter_context(tc.tile_pool(name="sq", bufs=3))
        psum_pool = ctx.enter_context(tc.tile_pool(name="psum", bufs=8, space="PSUM"))
        stage_pool = ctx.enter_context(tc.tile_pool(name="stage", bufs=6))

        # ---------------- persistent tiles ----------------
        xnp = [
            const_pool.tile([P, HP, HP], BF16, name=f"xnp{hf}") for hf in range(NHF)
        ]
        sum_slots = const_pool.tile([P, NHF, NCHUNK], F32, name="sum_slots")
        ssq_slots = const_pool.tile([P, NHF, NCHUNK], F32, name="ssq_slots")

        sum_f = const_pool.tile([P, NHF], F32, name="sum_f")
        ssq_f = const_pool.tile([P, NHF], F32, name="ssq_f")
        mean_ch = const_pool.tile([P, NHF], F32, name="mean_ch")
        mean_bf = const_pool.tile([P, NHF], BF16, name="mean_bf")
        scale_ch = const_pool.tile([P, NHF], F32, name="scale_ch")
        m2_t = const_pool.tile([P, NHF], F32, name="m2_t")
        bias_ch = const_pool.tile([P, NHF], F32, name="bias_ch")
        biasp_ch = const_pool.tile([P, NHF], F32, name="biasp_ch")

        # group-layout weights (partition = group)
        dwg = const_pool.tile([P, 2, 18], F32, name="dwg")      # [g, m, (j,t)]
        pwg = const_pool.tile([P, 2, 2], F32, name="pwg")       # [g, o, i]
        weff = const_pool.tile([P, 2, 18], F32, name="weff")    # [g, o, (j,t)]
        weff_bf = const_pool.tile([P, 2, 18], BF16, name="weff_bf")
        u = const_pool.tile([P, NHF, 2, 9], F32, name="u")      # [ci, hf, o, t]
        b_row = const_pool.tile([1, C], F32, name="b_row")
        b_row_bf = const_pool.tile([1, C], BF16, name="b_row_bf")
        ones_f = const_pool.tile([1, 1], BF16, name="ones_f")

        # on-chip const generation
        pm4 = const_pool.tile([P, 2, 2, P], F32, name="pm4")    # iota for permTJ
        permTJ = const_pool.tile([P, 2, 2, P], BF16, name="permTJ")
        a2 = const_pool.tile([P, P // 2], F32, name="a2")       # 2*cp - ci
        tmp64 = const_pool.tile([P, P // 2], F32, name="tmp64")
        red1 = const_pool.tile([P, 1], F32, name="red1")
        par_f = const_pool.tile([P, 1], F32, name="par_f")
        maskh = const_pool.tile([P, P // 2], F32, name="maskh")
        ones_c = const_pool.tile([P, 1], F32, name="ones_c")

        it128 = const_pool.tile([P, P], F32, name="it128")
        masks2 = const_pool.tile([P, 2, P], F32, name="masks2")
        zz18 = const_pool.tile([P, 18], F32, name="zz18")

        # dense block-diag weights: raw bf16 (unscaled) and scaled bf16
        lhsT_raw = const_pool.tile([P, NHF, 9, P], BF16, name="lhsT_raw")
        lhsT_sb = const_pool.tile([P, NHF, 9, P], BF16, name="lhsT_sb")

        zz = const_pool.tile([P, 1], F32, name="zz")
        zz2 = const_pool.tile([P, 1], F32, name="zz2")
        zz_bf = const_pool.tile([P, 512], BF16, name="zz_bf")

        # prewarm ScalarE LUT tables off the critical path
        with tc.high_priority():
            nc.vector.memset(zz[:], 0.0)
            nc.scalar.sqrt(zz2[:], zz[:])
            nc.scalar.activation(
                out=zz2[:], in_=zz[:], func=ACTF.Identity, bias=zz[:], scale=0.0
            )
            nc.scalar.activation(out=zz2[:], in_=zz[:], func=ACTF.Square)
            nc.vector.memset(zz_bf[:], 0.0)
            nc.vector.memset(ones_c[:], 1.0)
            nc.vector.memset(ones_f[:], 1.0)

        # ------------- input DMAs: sync + gpsimd rings -------------
        def emit_chunk(hf, ck, eng):
            r0, r1 = CK[ck]
            eng.dma_start(
                out=xnp[hf][:, r0:r1, :],
                in_=x_ext[hf * P : (hf + 1) * P, r0:r1, :],
            )

        with tc.high_priority():
            # const-gen iotas first on gpsimd (cheap), then the tiny weight
            # DMAs on its ring: their slow small-packet drain must not block
            # the sync/scalar ring heads where the first x chunks ride
            nc.gpsimd.iota(
                pm4[:], pattern=[[128, 2], [1, 2], [1, P]], base=-1,
                channel_multiplier=-2, allow_small_or_imprecise_dtypes=True,
            )
            nc.gpsimd.iota(
                a2[:], pattern=[[2, P // 2]], base=0, channel_multiplier=-1,
                allow_small_or_imprecise_dtypes=True,
            )
            nc.gpsimd.dma_start(
                out=dwg[:],
                in_=bass.AP(tensor=dw_ext, offset=0, ap=[[36, P], [1, 36]]),
            )
            nc.gpsimd.dma_start(
                out=pwg[:],
                in_=bass.AP(tensor=pw_ext, offset=0, ap=[[4, P], [1, 4]]),
            )
            nc.gpsimd.dma_start(
                out=b_row[:], in_=bass.AP(tensor=b_ext, offset=0, ap=[[C, 1], [1, C]])
            )
            # all three queues carry h0 first, then h1 (ring FIFO keeps
            # the priority); alternate emission for DMA sem-lane distance
            Q3 = (nc.sync, nc.scalar, nc.gpsimd)
            for ck in range(NCHUNK):
                emit_chunk(0, ck, Q3[ck % 3])
            for ck in range(NCHUNK):
                emit_chunk(1, ck, Q3[(ck + 2) % 3])

        # ------------- DVE const chain (cheap, before chunks land) -------------
        # permTJ[g, hf, 1-j, p] = (128hf + (1-j) + p - 1 - 2g == 0)
        #                       = (g == 64hf + p//2 and p%2 == j)
        nc.vector.tensor_scalar(
            out=permTJ[:], in0=pm4[:], scalar1=0.0, scalar2=None, op0=OP.is_equal
        )
        # par_f[p] = p % 2  (via  1 - sum_cp [2cp - p == 0])
        nc.vector.tensor_scalar(
            out=tmp64[:], in0=a2[:], scalar1=0.0, scalar2=None, op0=OP.is_equal
        )
        nc.vector.tensor_reduce(out=red1[:], in_=tmp64[:], axis=AX.X, op=OP.add)
        nc.vector.tensor_scalar(
            out=par_f[:], in0=red1[:], scalar1=-1.0, scalar2=1.0,
            op0=OP.mult, op1=OP.add,
        )
        # maskh[ci, cp] = (cp == ci // 2)  <=>  (2cp - ci + ci%2 == 0)
        if _F_NO_STRIDE0:
            nc.vector.tensor_scalar(
                out=tmp64[:], in0=a2[:], scalar1=par_f[:, 0:1], scalar2=None,
                op0=OP.add,
            )
            nc.gpsimd.iota(it128[:], pattern=[[1, P]], base=0,
                           channel_multiplier=-1,
                           allow_small_or_imprecise_dtypes=True)
            nc.vector.tensor_scalar(
                out=masks2[:, 0, :], in0=it128[:], scalar1=par_f[:, 0:1],
                scalar2=None, op0=OP.add,
            )
            nc.vector.tensor_scalar(
                out=masks2[:, 1, :], in0=masks2[:, 0, :], scalar1=1.0,
                scalar2=None, op0=OP.is_equal,
            )
            nc.vector.tensor_scalar(
                out=masks2[:, 0, :], in0=masks2[:, 0, :], scalar1=0.0,
                scalar2=None, op0=OP.is_equal,
            )
            nc.vector.memset(zz18[:], 0.0)
        else:
            nc.vector.scalar_tensor_tensor(
                out=tmp64[:],
                in0=ones_c[:, 0:1].broadcast_to([P, P // 2]),
                scalar=par_f[:, 0:1],
                in1=a2[:],
                op0=OP.mult,
                op1=OP.add,
            )
        nc.vector.tensor_scalar(
            out=maskh[:], in0=tmp64[:], scalar1=0.0, scalar2=None, op0=OP.is_equal
        )

        # ------------- stats helpers -------------
        def emit_sum_dve(hf, ck):
            r0, r1 = CK[ck]
            return nc.vector.tensor_reduce(
                out=sum_slots[:, hf, ck : ck + 1],
                in_=xnp[hf][:, r0:r1, :],
                axis=AX.XY,
                op=OP.add,
            )

        def emit_sum_gps(hf, ck):
            r0, r1 = CK[ck]
            scr = sq_pool.tile([P, 18, HP], BF16, name="gscr")
            nc.gpsimd.tensor_scalar(
                out=scr[:, : r1 - r0, :],
                in0=xnp[hf][:, r0:r1, :],
                scalar1=1.0,
                scalar2=None,
                op0=OP.mult,
                op1=OP.add,
                accum_out=sum_slots[:, hf, ck : ck + 1],
            )

        def emit_sq_act(hf, ck):
            r0, r1 = CK[ck]
            sq = sq_pool.tile([P, 18, HP], BF16, name="sq")
            return nc.scalar.activation(
                out=sq[:, : r1 - r0, :],
                in_=xnp[hf][:, r0:r1, :],
                func=ACTF.Square,
                accum_out=ssq_slots[:, hf, ck : ck + 1],
            )

        def emit_sum_dve(hf, ck):
            r0, r1 = CK[ck]
            return nc.vector.tensor_reduce(
                out=sum_slots[:, hf, ck : ck + 1],
                in_=xnp[hf][:, r0:r1, :],
                axis=AX.XY,
                op=OP.add,
            )

        def emit_sum_act(hf, ck):
            r0, r1 = CK[ck]
            scr = sq_pool.tile([P, 18, HP], BF16, name="iscr")
            return nc.scalar.activation(
                out=scr[:, : r1 - r0, :],
                in_=xnp[hf][:, r0:r1, :],
                func=ACTF.Identity,
                bias=zz[:],
                scale=1.0,
                accum_out=sum_slots[:, hf, ck : ck + 1],
            )

        def emit_sq_dve(hf, ck):
            r0, r1 = CK[ck]
            sq = sq_pool.tile([P, 18, HP], BF16, name="sq")
            nc.vector.tensor_tensor(
                out=sq[:, : r1 - r0, :],
                in0=xnp[hf][:, r0:r1, :],
                in1=xnp[hf][:, r0:r1, :],
                op=OP.mult,
            )
            nc.vector.tensor_reduce(
                out=ssq_slots[:, hf, ck : ck + 1],
                in_=sq[:, : r1 - r0, :],
                axis=AX.XY,
                op=OP.add,
            )

        # ------------- stats: both halves fully before the conv -------------
        # (concurrent stats measurably slow the PE stream, so the conv
        # window is kept clean)
        # ------------- stats finalize + weight scale + borders -------------
        def emit_fin(hf):
            nc.vector.tensor_reduce(
                out=sum_f[:, hf : hf + 1],
                in_=sum_slots[:, hf, :],
                axis=AX.X,
                op=OP.add,
            )
            nc.vector.tensor_reduce(
                out=ssq_f[:, hf : hf + 1],
                in_=ssq_slots[:, hf, :],
                axis=AX.X,
                op=OP.add,
            )
            nc.vector.tensor_scalar(
                out=mean_ch[:, hf : hf + 1],
                in0=sum_f[:, hf : hf + 1],
                scalar1=1.0 / NPIX,
                scalar2=None,
                op0=OP.mult,
            )
            nc.vector.tensor_copy(mean_bf[:, hf : hf + 1], mean_ch[:, hf : hf + 1])
            nc.vector.tensor_tensor(
                out=m2_t[:, hf : hf + 1],
                in0=mean_ch[:, hf : hf + 1],
                in1=mean_ch[:, hf : hf + 1],
                op=OP.mult,
            )
            nc.vector.scalar_tensor_tensor(
                out=m2_t[:, hf : hf + 1],
                in0=m2_t[:, hf : hf + 1],
                scalar=float(-NPIX),
                in1=ssq_f[:, hf : hf + 1],
                op0=OP.mult,
                op1=OP.add,
            )
            nc.vector.tensor_scalar(
                out=m2_t[:, hf : hf + 1],
                in0=m2_t[:, hf : hf + 1],
                scalar1=1.0 / (NPIX - 1),
                scalar2=None,
                op0=OP.mult,
            )
            nc.scalar.sqrt(m2_t[:, hf : hf + 1], m2_t[:, hf : hf + 1])
            nc.vector.tensor_scalar(
                out=m2_t[:, hf : hf + 1],
                in0=m2_t[:, hf : hf + 1],
                scalar1=EPS,
                scalar2=None,
                op0=OP.add,
            )
            nc.vector.reciprocal(scale_ch[:, hf : hf + 1], m2_t[:, hf : hf + 1])
            # scale + cast the block-diag weights (per-partition ci)
            return nc.vector.tensor_scalar(
                out=lhsT_sb[:, hf],
                in0=lhsT_raw[:, hf],
                scalar1=scale_ch[:, hf : hf + 1],
                scalar2=None,
                op0=OP.mult,
            )

        BORDERS = (
            ((0, slice(None)), (1, slice(None))),
            ((HP - 1, slice(None)), (1, slice(None))),
            ((slice(1, 1 + H), 0), (slice(1, 1 + H), 1)),
            ((slice(1, 1 + H), HP - 1), (slice(1, 1 + H), 1)),
        )

        def emit_borders_act(hf):
            last = None
            for dst, src in BORDERS:
                last = nc.scalar.activation(
                    out=xnp[hf][:, dst[0], dst[1]],
                    in_=xnp[hf][:, src[0], src[1]],
                    func=ACTF.Identity,
                    bias=mean_ch[:, hf : hf + 1],
                    scale=0.0,
                )
            return last

        def emit_borders_dve(hf):
            for dst, src in BORDERS:
                nc.vector.tensor_scalar(
                    out=xnp[hf][:, dst[0], dst[1]],
                    in0=xnp[hf][:, src[0], src[1]],
                    scalar1=0.0,
                    scalar2=mean_ch[:, hf : hf + 1],
                    op0=OP.mult,
                    op1=OP.add,
                )


        sum_insts = []

        def emit_stat_pair(hf, ck):
            sum_insts.append(emit_sum_dve(hf, ck))
            emit_sq_act(hf, ck)

        # first h0 chunks keep the DVE/ACT streams busy while the slow
        # small-packet weight DMA lands; the weight path is emitted after
        # them so it cannot stall the engine streams
        for ck in range(3):
            emit_stat_pair(0, ck)

        # ------------- weff (group layout): weff[g,o,:] = sum_q pw[g,o,q]*dw[g,q,:]
        for o in range(2):
            nc.vector.scalar_tensor_tensor(
                out=weff[:, o, :],
                in0=dwg[:, 0, :],
                scalar=pwg[:, o, 0:1],
                in1=zz18[:] if _F_NO_STRIDE0 else zz[:, 0:1].broadcast_to([P, 18]),
                op0=OP.mult,
                op1=OP.add,
            )
            nc.vector.scalar_tensor_tensor(
                out=weff[:, o, :],
                in0=dwg[:, 1, :],
                scalar=pwg[:, o, 1:2],
                in1=weff[:, o, :],
                op0=OP.mult,
                op1=OP.add,
            )
        nc.vector.tensor_copy(weff_bf[:], weff[:])

        # bias redistribution: bias_ch[p, hf] = b[128hf + p] via K=1 matmuls
        nc.vector.tensor_copy(b_row_bf[:], b_row[:])
        biasps = psum_pool.tile([P, NHF], F32, name="biasps", tag="ps", bufs=8)
        for hf in range(NHF):
            nc.tensor.matmul(
                biasps[:, hf : hf + 1],
                lhsT=b_row_bf[0:1, hf * P : (hf + 1) * P],
                rhs=ones_f[0:1, 0:1],
                start=True,
                stop=True,
            )
        nc.vector.tensor_copy(bias_ch[:], biasps[:])

        # u[p, hf, o, t] = weff[64hf + p//2, o, p%2, t] via 2 accumulated
        # permutation matmuls per half (parity is folded into permTJ)
        ups = psum_pool.tile([P, NHF, 18], F32, name="ups", tag="ps", bufs=8)
        for hf in range(NHF):
            for j in range(2):
                nc.tensor.matmul(
                    ups[:, hf, :],
                    lhsT=permTJ[:, hf, 1 - j, :],
                    rhs=weff_bf[:, :, 9 * j : 9 * (j + 1)],
                    start=(j == 0),
                    stop=(j == 1),
                )
        nc.vector.tensor_copy(u[:], ups[:])

        # ------------- dense block-diag build: one broadcast TT per half ------
        # lhsT_raw[ci, hf, t, 2cp+e] = maskh[ci, cp] * u[ci, hf, e, t]
        def emit_build(hf):
            if _F_NO_STRIDE0:
                for t in range(9):
                    nc.scalar.activation(
                        out=lhsT_raw[:, hf, t, :],
                        in_=masks2[:, 0, :],
                        func=ACTF.Identity,
                        bias=zz[:],
                        scale=u[:, hf, 0, t : t + 1],
                    )
                    nc.vector.scalar_tensor_tensor(
                        out=lhsT_raw[:, hf, t, :],
                        in0=masks2[:, 1, :],
                        scalar=u[:, hf, 1, t : t + 1],
                        in1=lhsT_raw[:, hf, t, :],
                        op0=OP.mult,
                        op1=OP.add,
                    )
                return
            out_v = lhsT_raw[:, hf].rearrange("p t (c e) -> p t c e", e=2)
            in0_v = maskh[:, None, :, None].broadcast_to([P, 9, P // 2, 2])
            in1_v = (
                u[:, hf]
                .rearrange("p o t -> p t o")[:, :, None, :]
                .broadcast_to([P, 9, P // 2, 2])
            )
            nc.gpsimd.tensor_tensor(out=out_v, in0=in0_v, in1=in1_v, op=OP.mult)

        emit_build(0)
        emit_build(1)


        for ck in range(3, NCHUNK):
            emit_stat_pair(0, ck)
        for ck in range(6):
            emit_stat_pair(1, ck)
        with tc.high_priority(offset=2000):
            emit_fin(0)
            emit_borders_act(0)
        for ck in range(6, NCHUNK):
            emit_stat_pair(1, ck)

        # ------------- PE warm burst gated on last h0 chunk -------------
        wps0 = psum_pool.tile([P, 512], F32, name="wps", tag="ps", bufs=8)
        nc.tensor.matmul(
            wps0[:],
            lhsT=zz_bf[:, 0:P],
            rhs=xnp[1][:, 113:117, 1 : 1 + W],
            start=True,
            stop=True,
        )
        for _ in range(12):
            wps = psum_pool.tile([P, 512], F32, name="wps", tag="ps", bufs=8)
            nc.tensor.matmul(
                wps[:], lhsT=zz_bf[:, 0:P], rhs=zz_bf[:], start=True, stop=True
            )
        # second, shorter burst right before the conv becomes ready, so the
        # HAM activity window is warm when the first conv matmul issues
        wps2 = psum_pool.tile([P, 512], F32, name="wps", tag="ps", bufs=8)
        burst2 = nc.tensor.matmul(
            wps2[:], lhsT=zz_bf[:, 0:P], rhs=zz_bf[:], start=True, stop=True
        )
        bass._add_dep_helper(
            burst2.ins,
            sum_insts[14].ins,
            sync=True,
            reason="re-warm PE clock shortly before conv start",
        )
        for _ in range(5):
            wps = psum_pool.tile([P, 512], F32, name="wps", tag="ps", bufs=8)
            nc.tensor.matmul(
                wps[:], lhsT=zz_bf[:, 0:P], rhs=zz_bf[:], start=True, stop=True
            )

        with tc.high_priority(offset=2000):
            emit_fin(1)
            emit_borders_dve(1)

        # ------------- conv + epilogue -------------
        OUT_ENGS = (nc.gpsimd,) if _F_OUT_GPSIMD else (nc.sync, nc.scalar, nc.gpsimd)
        stage_idx = [0]

        def emit_conv_mms(hf, sb):
            ps = [
                psum_pool.tile([P, ROWS_PER_MM, W], F32, name="ps", tag="ps", bufs=8)
                for _ in range(SB_TILES)
            ]
            for t in range(9):
                dy, dx = t // 3, t % 3
                for k in range(SB_TILES):
                    h0 = sb * SB_ROWS + k * ROWS_PER_MM
                    nc.tensor.matmul(
                        ps[k][:],
                        lhsT=lhsT_sb[:, hf, t, :],
                        rhs=xnp[hf][
                            :, h0 + dy : h0 + dy + ROWS_PER_MM, dx : dx + W
                        ],
                        start=(t == 0),
                        stop=(t == 8),
                    )
            return ps

        def emit_conv_epi(hf, sb, ps, n_stages=2):
            blocks_per_stage = SB_TILES // n_stages
            for stg_i in range(n_stages):
                rows = blocks_per_stage * ROWS_PER_MM
                stg = stage_pool.tile([P, rows, W], BF16, name="stg")
                for kk in range(blocks_per_stage):
                    k = stg_i * blocks_per_stage + kk
                    nc.scalar.activation(
                        out=stg[:, kk * ROWS_PER_MM : (kk + 1) * ROWS_PER_MM, :],
                        in_=ps[k][:],
                        func=ACTF.Identity,
                        bias=biasp_ch[:, hf : hf + 1],
                        scale=1.0,
                    )
                r0 = sb * SB_ROWS + stg_i * rows
                if n_stages == 4:
                    eng = (nc.sync, nc.scalar)[stg_i % 2]
                else:
                    eng = OUT_ENGS[stage_idx[0] % len(OUT_ENGS)]
                stage_idx[0] += 1
                eng.dma_start(
                    out=out_ext[hf * P : (hf + 1) * P, r0 : r0 + rows, :],
                    in_=stg[:],
                )

        def emit_conv(hf, sb, n_stages=2):
            emit_conv_epi(hf, sb, emit_conv_mms(hf, sb), n_stages)

        def emit_bias(hf):
            bps = psum_pool.tile([P, 1], F32, name="bps", tag="ps", bufs=8)
            for t in range(9):
                nc.tensor.matmul(
                    bps[:],
                    lhsT=lhsT_sb[:, hf, t, :],
                    rhs=mean_bf[:, hf : hf + 1],
                    start=(t == 0),
                    stop=(t == 8),
                )
            with tc.high_priority(offset=2000):
                nc.vector.tensor_tensor(
                    out=biasp_ch[:, hf : hf + 1],
                    in0=bias_ch[:, hf : hf + 1],
                    in1=bps[:],
                    op=OP.subtract,
                )

        ps0 = emit_conv_mms(0, 0)
        emit_bias(0)  # PE runs these right after sb0's matmuls
        emit_conv_epi(0, 0, ps0)

        for sb in range(1, NSB):
            emit_conv(0, sb)
        emit_bias(1)
        for sb in range(NSB):
            if sb == NSB - 1:
                emit_conv(1, sb, n_stages=4)
            else:
                emit_conv(1, sb)

    nc.compile()
    return nc


def get_nc():
    if "nc" not in _CACHED:
        _CACHED["nc"] = build_nc()
    return _CACHED["nc"]


def make_in_maps(x, dw_kernels, pw_kernels, biases):
    x = np.asarray(x, dtype=np.float32)
    dw_kernels = np.asarray(dw_kernels, dtype=np.float32)
    pw_kernels = np.asarray(pw_kernels, dtype=np.float32)
    biases = np.asarray(biases, dtype=np.float32)
    B = x.shape[0]
    xp = np.zeros((B, C, HP, HP), dtype=ml_dtypes.bfloat16)
    xp[:, :, 1 : 1 + H, 1 : 1 + W] = x.astype(ml_dtypes.bfloat16)
    return [
        {
            "x": np.ascontiguousarray(xp[i]),
            "dw_kernels": np.ascontiguousarray(dw_kernels[i]),
            "pw_kernels": np.ascontiguousarray(pw_kernels[i]),
            "biases": np.ascontiguousarray(biases[i]),
        }
        for i in range(B)
    ]


def kernel(x, dw_kernels, pw_kernels, biases):
    B = np.asarray(x).shape[0]
    assert B == 8
    nc = get_nc()
    in_maps = make_in_maps(x, dw_kernels, pw_kernels, biases)
    res = run_bass_kernel_spmd(nc, in_maps, core_ids=list(range(B)))
    return np.stack(
        [np.asarray(res.results[i]["out"]).astype(np.float32) for i in range(B)],
        axis=0,
    )


# revision 33
# speedup vs baseline: 1.0093x; 1.0093x over previous
"""AdaConv2D Trainium2 Bass kernel.

Problem (per sample): instance-norm(x) -> grouped 3x3 conv (128 groups,
2ch/group, per-sample weights) -> grouped 1x1 conv -> +bias.
B=8, Cin=Cout=256, H=W=128.

Strategy: pure data-parallel, 1 sample per NeuronCore (8 cores).

Per-core design (v3):
  - Host pads x to [256, 130, 130] bf16 with zero borders.  Each input
    chunk DMA lands *directly* in the padded SBUF image xnp (one
    contiguous descriptor per partition) — no on-chip ingest copy.
    Input rides the sync + gpsimd queues (the scalar engine is kept free
    for ACT stats so its DMA issues never stall behind compute).
  - Stats (per half, 8 chunks): per-channel sum via tensor_reduce-XY
    into per-chunk slots (DVE x5, GpSimd tensor_scalar+accum x2, the
    DVE also handles the last chunk); sum-of-squares via ACT
    Square+accum (7 chunks) and DVE TT-square+reduce (last chunk).
    Pad zeros don't disturb the sums.  DVE reductions are 1x by HW
    ucode (PERF_ONE), so the work is spread over three engines.
  - The 1x1 conv is folded into the 3x3 weights; instance norm is folded
    into weights + bias (border cells hold the per-channel mean so
    (border - mean) * scale = 0 matches the reference's zero padding).
  - Weights: 2 PSUM-accumulated permutation matmuls per half move
    group-layout weights to channel-partition layout (parity folded into
    the permutation); one broadcast tensor_tensor per half expands them
    into the dense block-diagonal lhsT (bf16); a tensor_scalar applies
    the norm scale after stats resolve.  h0's finalize+scale chain is
    high-priority and h1's DVE work is dependency-pinned behind it so
    the scheduler cannot interleave h1 into the critical path.
  - Conv: 9 shifted block-diagonal 128x128 bf16 matmuls per 4-row PSUM
    tile, channels on partitions (2 halves).  A dummy-matmul burst gated
    on the last h0 chunk warms the PE clock (HAM) so the conv streams at
    2.4 GHz.
  - Output staged bf16 (host converts back to f32); out-DMAs round-robin
    across sync/scalar/gpsimd queues; the last superblock drains in
    4-row stages on the two HWDGE queues to shorten the tail.
"""

import os
import sys

sys.path.insert(0, "/opt/trn_rl_repo")

from contextlib import ExitStack

# bisection flags (default 0 = full-speed path)
_F_ALL_ACT_SQ = os.environ.get("ADK_ALL_ACT_SQ", "0") == "1"
_F_OUT_GPSIMD = os.environ.get("ADK_OUT_GPSIMD", "0") == "1"
_F_NO_STRIDE0 = os.environ.get("ADK_NO_STRIDE0", "0") == "1"
_F_NO_GPS_SUM = os.environ.get("ADK_NO_GPS_SUM", "0") == "1"

import numpy as np
import ml_dtypes

from concourse import bacc, bass, mybir, tile
from concourse.bass_utils import run_bass_kernel_spmd

F32 = mybir.dt.float32
BF16 = mybir.dt.bfloat16
AX = mybir.AxisListType
OP = mybir.AluOpType
ACTF = mybir.ActivationFunctionType

C = 256          # channels (per sample)
H = W = 128      # spatial
P = 128          # partitions
HP = H + 2       # padded rows/cols (130)
NHF = 2          # channel halves
NPIX = H * W
EPS = 1e-7

# padded-row chunk splits; small leading chunks start the stats
# pipeline as early as possible
CK = [(0, 8), (8, 16), (16, 32), (32, 48), (48, 64), (64, 80),
      (80, 96), (96, 112), (112, 130)]
NCHUNK = len(CK)

ROWS_PER_MM = 4                   # output rows per psum tile (4*128=512)
SB_TILES = 4                      # psum tiles per superblock
SB_ROWS = ROWS_PER_MM * SB_TILES  # 16 rows per superblock
NSB = H // SB_ROWS                # 8 superblocks per half

_CACHED = {}


def build_nc():
    nc = bacc.Bacc(trn_type="TRN2")

    x_ext = nc.declare_dram_parameter("x", [C, HP, HP], BF16, isOutput=False)
    dw_ext = nc.declare_dram_parameter("dw_kernels", [C, 2, 3, 3], F32, isOutput=False)
    pw_ext = nc.declare_dram_parameter("pw_kernels", [C, 2, 1, 1], F32, isOutput=False)
    b_ext = nc.declare_dram_parameter("biases", [C], F32, isOutput=False)
    out_ext = nc.declare_dram_parameter("out", [C, H, W], BF16, isOutput=True)

    with tile.TileContext(nc) as tc, ExitStack() as ctx:
        const_pool = ctx.enter_context(tc.tile_pool(name="const", bufs=1))
        sq_pool = ctx.enter_context(tc.tile_pool(name="sq", bufs=3))
        psum_pool = ctx.enter_context(tc.tile_pool(name="psum", bufs=8, space="PSUM"))
        stage_pool = ctx.enter_context(tc.tile_pool(name="stage", bufs=6))

        # ---------------- persistent tiles ----------------
        xnp = [
            const_pool.tile([P, HP, HP], BF16, name=f"xnp{hf}") for hf in range(NHF)
        ]
        sum_slots = const_pool.tile([P, NHF, NCHUNK], F32, name="sum_slots")
        ssq_slots = const_pool.tile([P, NHF, NCHUNK], F32, name="ssq_slots")

        sum_f = const_pool.tile([P, NHF], F32, name="sum_f")
        ssq_f = const_pool.tile([P, NHF], F32, name="ssq_f")
        mean_ch = const_pool.tile([P, NHF], F32, name="mean_ch")
        mean_bf = const_pool.tile([P, NHF], BF16, name="mean_bf")
        scale_ch = const_pool.tile([P, NHF], F32, name="scale_ch")
        m2_t = const_pool.tile([P, NHF], F32, name="m2_t")
        bias_ch = const_pool.tile([P, NHF], F32, name="bias_ch")
        biasp_ch = const_pool.tile([P, NHF], F32, name="biasp_ch")

        # group-layout weights (partition = group)
        dwg = const_pool.tile([P, 2, 18], F32, name="dwg")      # [g, m, (j,t)]
        pwg = const_pool.tile([P, 2, 2], F32, name="pwg")       # [g, o, i]
        weff = const_pool.tile([P, 2, 18], F32, name="weff")    # [g, o, (j,t)]
        weff_bf = const_pool.tile([P, 2, 18], BF16, name="weff_bf")
        u = const_pool.tile([P, NHF, 2, 9], F32, name="u")      # [ci, hf, o, t]
        b_row = const_pool.tile([1, C], F32, name="b_row")
        b_row_bf = const_pool.tile([1, C], BF16, name="b_row_bf")
        ones_f = const_pool.tile([1, 1], BF16, name="ones_f")

        # on-chip const generation
        pm4 = const_pool.tile([P, 2, 2, P], F32, name="pm4")    # iota for permTJ
        permTJ = const_pool.tile([P, 2, 2, P], BF16, name="permTJ")
        a2 = const_pool.tile([P, P // 2], F32, name="a2")       # 2*cp - ci
        tmp64 = const_pool.tile([P, P // 2], F32, name="tmp64")
        red1 = const_pool.tile([P, 1], F32, name="red1")
        par_f = const_pool.tile([P, 1], F32, name="par_f")
        maskh = const_pool.tile([P, P // 2], F32, name="maskh")
        ones_c = const_pool.tile([P, 1], F32, name="ones_c")

        it128 = const_pool.tile([P, P], F32, name="it128")
        masks2 = const_pool.tile([P, 2, P], F32, name="masks2")
        zz18 = const_pool.tile([P, 18], F32, name="zz18")

        # dense block-diag weights: raw bf16 (unscaled) and scaled bf16
        lhsT_raw = const_pool.tile([P, NHF, 9, P], BF16, name="lhsT_raw")
        lhsT_sb = const_pool.tile([P, NHF, 9, P], BF16, name="lhsT_sb")

        zz = const_pool.tile([P, 1], F32, name="zz")
        zz2 = const_pool.tile([P, 1], F32, name="zz2")
        zz_bf = const_pool.tile([P, 512], BF16, name="zz_bf")

        # prewarm ScalarE LUT tables off the critical path
        with tc.high_priority():
            nc.vector.memset(zz[:], 0.0)
            nc.scalar.sqrt(zz2[:], zz[:])
            nc.scalar.activation(
                out=zz2[:], in_=zz[:], func=ACTF.Identity, bias=zz[:], scale=0.0
            )
            nc.scalar.activation(out=zz2[:], in_=zz[:], func=ACTF.Square)
            nc.vector.memset(zz_bf[:], 0.0)
            nc.vector.memset(ones_c[:], 1.0)
            nc.vector.memset(ones_f[:], 1.0)

        # ------------- input DMAs: sync + gpsimd rings -------------
        def emit_chunk(hf, ck, eng):
            r0, r1 = CK[ck]
            eng.dma_start(
                out=xnp[hf][:, r0:r1, :],
                in_=x_ext[hf * P : (hf + 1) * P, r0:r1, :],
            )

        with tc.high_priority():
            # const-gen iotas first on gpsimd (cheap), then the tiny weight
            # DMAs on its ring: their slow small-packet drain must not block
            # the sync/scalar ring heads where the first x chunks ride
            nc.gpsimd.iota(
                pm4[:], pattern=[[128, 2], [1, 2], [1, P]], base=-1,
                channel_multiplier=-2, allow_small_or_imprecise_dtypes=True,
            )
            nc.gpsimd.iota(
                a2[:], pattern=[[2, P // 2]], base=0, channel_multiplier=-1,
                allow_small_or_imprecise_dtypes=True,
            )
            nc.gpsimd.dma_start(
                out=dwg[:],
                in_=bass.AP(tensor=dw_ext, offset=0, ap=[[36, P], [1, 36]]),
            )
            nc.gpsimd.dma_start(
                out=pwg[:],
                in_=bass.AP(tensor=pw_ext, offset=0, ap=[[4, P], [1, 4]]),
            )
            nc.gpsimd.dma_start(
                out=b_row[:], in_=bass.AP(tensor=b_ext, offset=0, ap=[[C, 1], [1, C]])
            )
            # all three queues carry h0 first, then h1 (ring FIFO keeps
            # the priority); alternate emission for DMA sem-lane distance
            Q3 = (nc.sync, nc.scalar, nc.gpsimd)
            for ck in range(NCHUNK):
                emit_chunk(0, ck, Q3[ck % 3])
            for ck in range(NCHUNK):
                emit_chunk(1, ck, Q3[(ck + 2) % 3])

        # ------------- DVE const chain (cheap, before chunks land) -------------
        # permTJ[g, hf, 1-j, p] = (128hf + (1-j) + p - 1 - 2g == 0)
        #                       = (g == 64hf + p//2 and p%2 == j)
        nc.vector.tensor_scalar(
            out=permTJ[:], in0=pm4[:], scalar1=0.0, scalar2=None, op0=OP.is_equal
        )
        # par_f[p] = p % 2  (via  1 - sum_cp [2cp - p == 0])
        nc.vector.tensor_scalar(
            out=tmp64[:], in0=a2[:], scalar1=0.0, scalar2=None, op0=OP.is_equal
        )
        nc.vector.tensor_reduce(out=red1[:], in_=tmp64[:], axis=AX.X, op=OP.add)
        nc.vector.tensor_scalar(
            out=par_f[:], in0=red1[:], scalar1=-1.0, scalar2=1.0,
            op0=OP.mult, op1=OP.add,
        )
        # maskh[ci, cp] = (cp == ci // 2)  <=>  (2cp - ci + ci%2 == 0)
        if _F_NO_STRIDE0:
            nc.vector.tensor_scalar(
                out=tmp64[:], in0=a2[:], scalar1=par_f[:, 0:1], scalar2=None,
                op0=OP.add,
            )
            nc.gpsimd.iota(it128[:], pattern=[[1, P]], base=0,
                           channel_multiplier=-1,
                           allow_small_or_imprecise_dtypes=True)
            nc.vector.tensor_scalar(
                out=masks2[:, 0, :], in0=it128[:], scalar1=par_f[:, 0:1],
                scalar2=None, op0=OP.add,
            )
            nc.vector.tensor_scalar(
                out=masks2[:, 1, :], in0=masks2[:, 0, :], scalar1=1.0,
                scalar2=None, op0=OP.is_equal,
            )
            nc.vector.tensor_scalar(
                out=masks2[:, 0, :], in0=masks2[:, 0, :], scalar1=0.0,
                scalar2=None, op0=OP.is_equal,
            )
            nc.vector.memset(zz18[:], 0.0)
        else:
            nc.vector.scalar_tensor_tensor(
                out=tmp64[:],
                in0=ones_c[:, 0:1].broadcast_to([P, P // 2]),
                scalar=par_f[:, 0:1],
                in1=a2[:],
                op0=OP.mult,
                op1=OP.add,
            )
        nc.vector.tensor_scalar(
            out=maskh[:], in0=tmp64[:], scalar1=0.0, scalar2=None, op0=OP.is_equal
        )

        # ------------- stats helpers -------------
        def emit_sum_dve(hf, ck):
            r0, r1 = CK[ck]
            return nc.vector.tensor_reduce(
                out=sum_slots[:, hf, ck : ck + 1],
                in_=xnp[hf][:, r0:r1, :],
                axis=AX.XY,
                op=OP.add,
            )

        def emit_sum_gps(hf, ck):
            r0, r1 = CK[ck]
            scr = sq_pool.tile([P, 18, HP], BF16, name="gscr")
            nc.gpsimd.tensor_scalar(
                out=scr[:, : r1 - r0, :],
                in0=xnp[hf][:, r0:r1, :],
                scalar1=1.0,
                scalar2=None,
                op0=OP.mult,
                op1=OP.add,
                accum_out=sum_slots[:, hf, ck : ck + 1],
            )

        def emit_sq_act(hf, ck):
            r0, r1 = CK[ck]
            sq = sq_pool.tile([P, 18, HP], BF16, name="sq")
            return nc.scalar.activation(
                out=sq[:, : r1 - r0, :],
                in_=xnp[hf][:, r0:r1, :],
                func=ACTF.Square,
                accum_out=ssq_slots[:, hf, ck : ck + 1],
            )

        def emit_sum_dve(hf, ck):
            r0, r1 = CK[ck]
            return nc.vector.tensor_reduce(
                out=sum_slots[:, hf, ck : ck + 1],
                in_=xnp[hf][:, r0:r1, :],
                axis=AX.XY,
                op=OP.add,
            )

        def emit_sum_act(hf, ck):
            r0, r1 = CK[ck]
            scr = sq_pool.tile([P, 18, HP], BF16, name="iscr")
            return nc.scalar.activation(
                out=scr[:, : r1 - r0, :],
                in_=xnp[hf][:, r0:r1, :],
                func=ACTF.Identity,
                bias=zz[:],
                scale=1.0,
                accum_out=sum_slots[:, hf, ck : ck + 1],
            )

        def emit_sq_dve(hf, ck):
            r0, r1 = CK[ck]
            sq = sq_pool.tile([P, 18, HP], BF16, name="sq")
            nc.vector.tensor_tensor(
                out=sq[:, : r1 - r0, :],
                in0=xnp[hf][:, r0:r1, :],
                in1=xnp[hf][:, r0:r1, :],
                op=OP.mult,
            )
            nc.vector.tensor_reduce(
                out=ssq_slots[:, hf, ck : ck + 1],
                in_=sq[:, : r1 - r0, :],
                axis=AX.XY,
                op=OP.add,
            )

        # ------------- stats: both halves fully before the conv -------------
        # (concurrent stats measurably slow the PE stream, so the conv
        # window is kept clean)
        # ------------- stats finalize + weight scale + borders -------------
        def emit_fin(hf):
            nc.vector.tensor_reduce(
                out=sum_f[:, hf : hf + 1],
                in_=sum_slots[:, hf, :],
                axis=AX.X,
                op=OP.add,
            )
            nc.vector.tensor_reduce(
                out=ssq_f[:, hf : hf + 1],
                in_=ssq_slots[:, hf, :],
                axis=AX.X,
                op=OP.add,
            )
            nc.vector.tensor_scalar(
                out=mean_ch[:, hf : hf + 1],
                in0=sum_f[:, hf : hf + 1],
                scalar1=1.0 / NPIX,
                scalar2=None,
                op0=OP.mult,
            )
            nc.vector.tensor_copy(mean_bf[:, hf : hf + 1], mean_ch[:, hf : hf + 1])
            nc.vector.tensor_tensor(
                out=m2_t[:, hf : hf + 1],
                in0=mean_ch[:, hf : hf + 1],
                in1=mean_ch[:, hf : hf + 1],
                op=OP.mult,
            )
            nc.vector.scalar_tensor_tensor(
                out=m2_t[:, hf : hf + 1],
                in0=m2_t[:, hf : hf + 1],
                scalar=float(-NPIX),
                in1=ssq_f[:, hf : hf + 1],
                op0=OP.mult,
                op1=OP.add,
            )
            nc.vector.tensor_scalar(
                out=m2_t[:, hf : hf + 1],
                in0=m2_t[:, hf : hf + 1],
                scalar1=1.0 / (NPIX - 1),
                scalar2=None,
                op0=OP.mult,
            )
            nc.scalar.sqrt(m2_t[:, hf : hf + 1], m2_t[:, hf : hf + 1])
            nc.vector.tensor_scalar(
                out=m2_t[:, hf : hf + 1],
                in0=m2_t[:, hf : hf + 1],
                scalar1=EPS,
                scalar2=None,
                op0=OP.add,
            )
            nc.vector.reciprocal(scale_ch[:, hf : hf + 1], m2_t[:, hf : hf + 1])
            # scale + cast the block-diag weights (per-partition ci)
            return nc.vector.tensor_scalar(
                out=lhsT_sb[:, hf],
                in0=lhsT_raw[:, hf],
                scalar1=scale_ch[:, hf : hf + 1],
                scalar2=None,
                op0=OP.mult,
            )

        BORDERS = (
            ((0, slice(None)), (1, slice(None))),
            ((HP - 1, slice(None)), (1, slice(None))),
            ((slice(1, 1 + H), 0), (slice(1, 1 + H), 1)),
            ((slice(1, 1 + H), HP - 1), (slice(1, 1 + H), 1)),
        )

        def emit_borders_act(hf):
            last = None
            for dst, src in BORDERS:
                last = nc.scalar.activation(
                    out=xnp[hf][:, dst[0], dst[1]],
                    in_=xnp[hf][:, src[0], src[1]],
                    func=ACTF.Identity,
                    bias=mean_ch[:, hf : hf + 1],
                    scale=0.0,
                )
            return last

        def emit_borders_dve(hf):
            for dst, src in BORDERS:
                nc.vector.tensor_scalar(
                    out=xnp[hf][:, dst[0], dst[1]],
                    in0=xnp[hf][:, src[0], src[1]],
                    scalar1=0.0,
                    scalar2=mean_ch[:, hf : hf + 1],
                    op0=OP.mult,
                    op1=OP.add,
                )


        sum_insts = []

        def emit_stat_pair(hf, ck):
            sum_insts.append(emit_sum_dve(hf, ck))
            emit_sq_act(hf, ck)

        # first h0 chunks keep the DVE/ACT streams busy while the slow
        # small-packet weight DMA lands; the weight path is emitted after
        # them so it cannot stall the engine streams
        for ck in range(3):
            emit_stat_pair(0, ck)

        # ------------- weff (group layout): weff[g,o,:] = sum_q pw[g,o,q]*dw[g,q,:]
        for o in range(2):
            nc.vector.scalar_tensor_tensor(
                out=weff[:, o, :],
                in0=dwg[:, 0, :],
                scalar=pwg[:, o, 0:1],
                in1=zz18[:] if _F_NO_STRIDE0 else zz[:, 0:1].broadcast_to([P, 18]),
                op0=OP.mult,
                op1=OP.add,
            )
            nc.vector.scalar_tensor_tensor(
                out=weff[:, o, :],
                in0=dwg[:, 1, :],
                scalar=pwg[:, o, 1:2],
                in1=weff[:, o, :],
                op0=OP.mult,
                op1=OP.add,
            )
        nc.vector.tensor_copy(weff_bf[:], weff[:])

        # bias redistribution: bias_ch[p, hf] = b[128hf + p] via K=1 matmuls
        nc.vector.tensor_copy(b_row_bf[:], b_row[:])
        biasps = psum_pool.tile([P, NHF], F32, name="biasps", tag="ps", bufs=8)
        for hf in range(NHF):
            nc.tensor.matmul(
                biasps[:, hf : hf + 1],
                lhsT=b_row_bf[0:1, hf * P : (hf + 1) * P],
                rhs=ones_f[0:1, 0:1],
                start=True,
                stop=True,
            )
        nc.vector.tensor_copy(bias_ch[:], biasps[:])

        # u[p, hf, o, t] = weff[64hf + p//2, o, p%2, t] via 2 accumulated
        # permutation matmuls per half (parity is folded into permTJ)
        ups = psum_pool.tile([P, NHF, 18], F32, name="ups", tag="ps", bufs=8)
        for hf in range(NHF):
            for j in range(2):
                nc.tensor.matmul(
                    ups[:, hf, :],
                    lhsT=permTJ[:, hf, 1 - j, :],
                    rhs=weff_bf[:, :, 9 * j : 9 * (j + 1)],
                    start=(j == 0),
                    stop=(j == 1),
                )
        nc.vector.tensor_copy(u[:], ups[:])

        # ------------- dense block-diag build: one broadcast TT per half ------
        # lhsT_raw[ci, hf, t, 2cp+e] = maskh[ci, cp] * u[ci, hf, e, t]
        def emit_build(hf):
            if _F_NO_STRIDE0:
                for t in range(9):
                    nc.scalar.activation(
                        out=lhsT_raw[:, hf, t, :],
                        in_=masks2[:, 0, :],
                        func=ACTF.Identity,
                        bias=zz[:],
                        scale=u[:, hf, 0, t : t + 1],
                    )
                    nc.vector.scalar_tensor_tensor(
                        out=lhsT_raw[:, hf, t, :],
                        in0=masks2[:, 1, :],
                        scalar=u[:, hf, 1, t : t + 1],
                        in1=lhsT_raw[:, hf, t, :],
                        op0=OP.mult,
                        op1=OP.add,
                    )
                return
            out_v = lhsT_raw[:, hf].rearrange("p t (c e) -> p t c e", e=2)
            in0_v = maskh[:, None, :, None].broadcast_to([P, 9, P // 2, 2])
            in1_v = (
                u[:, hf]
                .rearrange("p o t -> p t o")[:, :, None, :]
                .broadcast_to([P, 9, P // 2, 2])
            )
            nc.gpsimd.tensor_tensor(out=out_v, in0=in0_v, in1=in1_v, op=OP.mult)

        emit_build(0)
        emit_build(1)


        for ck in range(3, NCHUNK):
            emit_stat_pair(0, ck)
        for ck in range(6):
            emit_stat_pair(1, ck)
        with tc.high_priority(offset=2000):
            emit_fin(0)
            emit_borders_act(0)
        for ck in range(6, NCHUNK):
            emit_stat_pair(1, ck)

        # ------------- PE warm burst gated on last h0 chunk -------------
        wps0 = psum_pool.tile([P, 512], F32, name="wps", tag="ps", bufs=8)
        nc.tensor.matmul(
            wps0[:],
            lhsT=zz_bf[:, 0:P],
            rhs=xnp[1][:, 113:117, 1 : 1 + W],
            start=True,
            stop=True,
        )
        for _ in range(12):
            wps = psum_pool.tile([P, 512], F32, name="wps", tag="ps", bufs=8)
            nc.tensor.matmul(
                wps[:], lhsT=zz_bf[:, 0:P], rhs=zz_bf[:], start=True, stop=True
            )
        # second, shorter burst right before the conv becomes ready, so the
        # HAM activity window is warm when the first conv matmul issues
        wps2 = psum_pool.tile([P, 512], F32, name="wps", tag="ps", bufs=8)
        burst2 = nc.tensor.matmul(
            wps2[:], lhsT=zz_bf[:, 0:P], rhs=zz_bf[:], start=True, stop=True
        )
        bass._add_dep_helper(
            burst2.ins,
            sum_insts[14].ins,
            sync=True,
            reason="re-warm PE clock shortly before conv start",
        )
        for _ in range(5):
            wps = psum_pool.tile([P, 512], F32, name="wps", tag="ps", bufs=8)
            nc.tensor.matmul(
                wps[:], lhsT=zz_bf[:, 0:P], rhs=zz_bf[:], start=True, stop=True
            )

        with tc.high_priority(offset=2000):
            emit_fin(1)
            emit_borders_dve(1)

        # ------------- conv + epilogue -------------
        OUT_ENGS = (nc.gpsimd,) if _F_OUT_GPSIMD else (nc.sync, nc.scalar, nc.gpsimd)
        stage_idx = [0]

        def emit_conv_mms(hf, sb):
            ps = [
                psum_pool.tile([P, ROWS_PER_MM, W], F32, name="ps", tag="ps", bufs=8)
                for _ in range(SB_TILES)
            ]
            for t in range(9):
                dy, dx = t // 3, t % 3
                for k in range(SB_TILES):
                    h0 = sb * SB_ROWS + k * ROWS_PER_MM
                    nc.tensor.matmul(
                        ps[k][:],
                        lhsT=lhsT_sb[:, hf, t, :],
                        rhs=xnp[hf][
                            :, h0 + dy : h0 + dy + ROWS_PER_MM, dx : dx + W
                        ],
                        start=(t == 0),
                        stop=(t == 8),
                    )
            return ps

        def emit_conv_epi(hf, sb, ps, n_stages=2):
            blocks_per_stage = SB_TILES // n_stages
            for stg_i in range(n_stages):
                rows = blocks_per_stage * ROWS_PER_MM
                stg = stage_pool.tile([P, rows, W], BF16, name="stg")
                for kk in range(blocks_per_stage):
                    k = stg_i * blocks_per_stage + kk
                    nc.scalar.activation(
                        out=stg[:, kk * ROWS_PER_MM : (kk + 1) * ROWS_PER_MM, :],
                        in_=ps[k][:],
                        func=ACTF.Identity,
                        bias=biasp_ch[:, hf : hf + 1],
                        scale=1.0,
                    )
                r0 = sb * SB_ROWS + stg_i * rows
                if n_stages == 4:
                    eng = (nc.sync, nc.scalar)[stg_i % 2]
                else:
                    eng = OUT_ENGS[stage_idx[0] % len(OUT_ENGS)]
                stage_idx[0] += 1
                eng.dma_start(
                    out=out_ext[hf * P : (hf + 1) * P, r0 : r0 + rows, :],
                    in_=stg[:],
                )

        def emit_conv(hf, sb, n_stages=2):
            emit_conv_epi(hf, sb, emit_conv_mms(hf, sb), n_stages)

        def emit_bias(hf):
            bps = psum_pool.tile([P, 1], F32, name="bps", tag="ps", bufs=8)
            for t in range(9):
                nc.tensor.matmul(
                    bps[:],
                    lhsT=lhsT_sb[:, hf, t, :],
                    rhs=mean_bf[:, hf : hf + 1],
                    start=(t == 0),
                    stop=(t == 8),
                )
            if hf == 0:
                with tc.high_priority(offset=2000):
                    nc.vector.tensor_tensor(
                        out=biasp_ch[:, hf : hf + 1],
                        in0=bias_ch[:, hf : hf + 1],
                        in1=bps[:],
                        op=OP.subtract,
                    )
            else:
                nc.vector.tensor_tensor(
                    out=biasp_ch[:, hf : hf + 1],
                    in0=bias_ch[:, hf : hf + 1],
                    in1=bps[:],
                    op=OP.subtract,
                )

        ps0 = emit_conv_mms(0, 0)
        emit_bias(0)  # PE runs these right after sb0's matmuls
        emit_conv_epi(0, 0, ps0)

        for sb in range(1, NSB):
            emit_conv(0, sb)
        emit_bias(1)
        for sb in range(NSB):
            if sb == NSB - 1:
                emit_conv(1, sb, n_stages=4)
            else:
                emit_conv(1, sb)

    nc.compile()
    return nc


def get_nc():
    if "nc" not in _CACHED:
        _CACHED["nc"] = build_nc()
    return _CACHED["nc"]


def make_in_maps(x, dw_kernels, pw_kernels, biases):
    x = np.asarray(x, dtype=np.float32)
    dw_kernels = np.asarray(dw_kernels, dtype=np.float32)
    pw_kernels = np.asarray(pw_kernels, dtype=np.float32)
    biases = np.asarray(biases, dtype=np.float32)
    B = x.shape[0]
    xp = np.zeros((B, C, HP, HP), dtype=ml_dtypes.bfloat16)
    xp[:, :, 1 : 1 + H, 1 : 1 + W] = x.astype(ml_dtypes.bfloat16)
    return [
        {
            "x": np.ascontiguousarray(xp[i]),
            "dw_kernels": np.ascontiguousarray(dw_kernels[i]),
            "pw_kernels": np.ascontiguousarray(pw_kernels[i]),
            "biases": np.ascontiguousarray(biases[i]),
        }
        for i in range(B)
    ]


def kernel(x, dw_kernels, pw_kernels, biases):
    B = np.asarray(x).shape[0]
    assert B == 8
    nc = get_nc()
    in_maps = make_in_maps(x, dw_kernels, pw_kernels, biases)
    res = run_bass_kernel_spmd(nc, in_maps, core_ids=list(range(B)))
    return np.stack(
        [np.asarray(res.results[i]["out"]).astype(np.float32) for i in range(B)],
        axis=0,
    )
